# revision 70
# baseline (speedup 1.0000x reference)
"""CoordAttention kernel for Trainium2 (8 NeuronCores, data-parallel over batch).

v3: engine-rebalanced. The original baseline was compute-bound: PE spent
148us on fp16 identity-matmul sum pools and DVE 166us on max trees + the
final multiplies + psum folds. Changes:
  - sum pools run as fp8e4 DoubleRow matmuls (PE ingests a row-pair per
    output column) against a doubled identity, accumulating each direction
    into one [128, W]/[128, H] psum tile; the means are read straight from
    PSUM by ACT, so DVE does no fold work. The fp8 copy of x is produced on
    the host and DMA'd (DMA has slack; ACT does not). fp8 quantization noise
    averages out in the means (~1e-4 on the attention weights; tol is 2e-2).
  - max trees stay on DVE in 2x mode (the only >=2 elem/cycle max on TRN2:
    gpsimd TT is rejected by walrus codegen, InstPool/TensorReduce run 1x).
    tree_w runs per row-half and tree_h's first stage is row-half aligned,
    so tree work starts when half a tile has landed.
  - the apply phase splits the s=a_h*a_c multiply between DVE (TT on packed
    pairs, 2x) and ACT (per-row activations with a per-partition scale),
    since ACT is otherwise idle while DVE is the bottleneck.
  - emission order is tuned for the per-engine in-order queues: sample 1's
    trees sit between sample 0's pools and sample 0's applies in the DVE
    stream, sample 1's means are deferred until after sample 0's apply so
    ACT never head-of-line blocks on sample 1's psums.

Reference computation (per sample, inference):
  pools:  mean/max over W -> [C,H];  mean/max over H -> [C,W];  global -> [C]
  x_cat = concat(pools) -> [C, H+W+1, 2]
  y = BN(conv1x1(x_cat)) ; y = h_swish(y)
  a_h = sigmoid(conv(xh)), a_w = sigmoid(conv(xw)), a_c = sigmoid(conv(xc))
  out = x * a_w * a_h * a_c
"""
import sys

for _p in ("/opt/trn_rl_repo", "/root/.axon_site/_ro/trn_rl_repo"):
    if _p not in sys.path:
        sys.path.insert(0, _p)

import copy as _copy
import numpy as np

import concourse.bass as bass
import concourse.mybir as mybir
import concourse.tile as tile

f32 = mybir.dt.float32
f16 = mybir.dt.float16
f8 = mybir.dt.float8e4
OP = mybir.AluOpType
AF = mybir.ActivationFunctionType
AX = mybir.AxisListType
DR = mybir.MatmulPerfMode.DoubleRow

N, C, H, W = 16, 256, 128, 128
TC = 8
NCORES = 8
PER = N // NCORES
EPS = 1e-5
CT = C // 128           # channel tiles per sample
SS = H + W + 1          # pooled sequence length
G8 = 8                  # first-stage groups in the max trees
HF = H // 2
RA = 48                 # rows per half-tile whose s-multiply stays on DVE;
                        # the remaining HF-RA rows go to ACT row-scales


def _split_excess_waits(nc, limit=1):
    """This container's walrus accepts only one sync-wait per instruction;
    hoist extras onto same-engine drain carriers inserted just before."""
    m = nc.m
    newm = _copy.replace(m, functions=[])
    for fn in m.functions:
        newfn = _copy.replace(fn, blocks=[])
        newfn.set_allocations_from_list(fn.allocations)
        for blk in fn.blocks:
            out = []
            for inst in blk.instructions:
                si = inst.sync_info
                waits = list(si.on_wait) if si and si.on_wait else []
                if len(waits) > limit:
                    keep, excess = waits[-limit:], waits[: len(waits) - limit]
                    for gi, wchunk in enumerate(excess):
                        d = mybir.InstDrain(
                            name=f"{inst.name}-wsplit{gi}", ins=[], outs=[]
                        )
                        d.engine = inst.engine
                        d.sync_info = mybir.SyncInfo(on_wait=[wchunk], on_update=[])
                        out.append(d)
                    inst.sync_info = mybir.SyncInfo(
                        on_wait=keep, on_update=list(si.on_update or [])
                    )
                out.append(inst)
            newfn.blocks.append(_copy.replace(blk, instructions=out))
        newm.functions.append(newfn)
    nc.m = newm


def build_nc(per=PER, xp_bufs=4, split_waits=True):
    # Recalibrate the tile scheduler's cost model to measured HW rates (the PE
    # streams fp16 moving data at ~0.83 ns/col on real silicon, DMA ~435 GB/s)
    # so the static per-engine order sequences the phases sensibly.
    # Scheduling decisions only; emitted instructions are unchanged.
    from concourse.hw_specs import TRN2Spec

    TRN2Spec.PE_CYCLE = TRN2Spec.PE_CYCLE_PSTATE_MID
    TRN2Spec.DMA_CYCLE = 1e9 / (435e9 / 128)
    # The model's DVE is ~2x pessimistic vs silicon (measured 580ns for a
    # 2048-elem 2x-mode TT vs ~1130ns modeled). Since the emitted semaphore
    # waits replay the model schedule, a slow-DVE model delays sample 0's
    # attention chain behind sample 1's sums. Calibrate to measured.
    TRN2Spec.CYCLE_T = {
        **TRN2Spec.CYCLE_T,
        mybir.EngineType.DVE: 1e9 / 1.9e9,
    }
    nc = bass.Bass()
    x_d = nc.declare_dram_parameter("x16", [per, C, H, W], f16, isOutput=False)
    x8_d = nc.declare_dram_parameter("x8", [per, C, H, W], f8, isOutput=False)
    o_d = nc.declare_dram_parameter("out", [per, C, H, W], f16, isOutput=True)
    id2_d = nc.declare_dram_parameter("id2", [128, 2, 128], f8, isOutput=False)
    w1_d = nc.declare_dram_parameter("w1t", [C, TC], f16, isOutput=False)
    b1_d = nc.declare_dram_parameter("b1f", [TC, 1], f32, isOutput=False)
    w2_d = nc.declare_dram_parameter("w2t", [TC, 2, C], f16, isOutput=False)
    w3_d = nc.declare_dram_parameter("w3t", [TC, 2, C], f16, isOutput=False)
    w4_d = nc.declare_dram_parameter("w4t", [TC, 2, C], f16, isOutput=False)
    b2_d = nc.declare_dram_parameter("b2r", [C, 1], f32, isOutput=False)
    b3_d = nc.declare_dram_parameter("b3r", [C, 1], f32, isOutput=False)
    b4_d = nc.declare_dram_parameter("b4r", [C, 1], f32, isOutput=False)

    def tree_w_rows(ch, out, scr, r0, r1):
        """max over w for rows r0:r1 (DVE): ch [128, H, W] -> out [128, r1-r0];
        scr [128, r1-r0, W//G8]."""
        gsz = W // G8
        nc.vector.tensor_tensor(
            out=scr,
            in0=ch[:, r0:r1, 0:gsz],
            in1=ch[:, r0:r1, gsz : 2 * gsz],
            op=OP.max,
        )
        for i in range(2, G8):
            nc.vector.tensor_tensor(
                out=scr, in0=scr, in1=ch[:, r0:r1, i * gsz : (i + 1) * gsz],
                op=OP.max,
            )
        g = gsz
        while g > 2:
            nc.vector.tensor_tensor(
                out=scr[:, :, 0 : g // 2],
                in0=scr[:, :, 0 : g // 2],
                in1=scr[:, :, g // 2 : g],
                op=OP.max,
            )
            g //= 2
        nc.vector.tensor_tensor(
            out=out, in0=scr[:, :, 0], in1=scr[:, :, 1], op=OP.max
        )

    def tree_h_first(ch, scr, hh):
        """fold row-half hh's four 16-row groups into scr [128, H//G8, W]."""
        gsz = H // G8
        base = hh * (G8 // 2)
        if hh == 0:
            nc.vector.tensor_tensor(
                out=scr,
                in0=ch[:, 0:gsz, :],
                in1=ch[:, gsz : 2 * gsz, :],
                op=OP.max,
            )
            rng = range(2, G8 // 2)
        else:
            rng = range(base, base + G8 // 2)
        for i in rng:
            nc.vector.tensor_tensor(
                out=scr, in0=scr, in1=ch[:, i * gsz : (i + 1) * gsz, :],
                op=OP.max,
            )

    def tree_h_tail(out, scr):
        g = H // G8
        while g > 2:
            nc.vector.tensor_tensor(
                out=scr[:, 0 : g // 2, :],
                in0=scr[:, 0 : g // 2, :],
                in1=scr[:, g // 2 : g, :],
                op=OP.max,
            )
            g //= 2
        nc.vector.tensor_tensor(
            out=out, in0=scr[:, 0, :], in1=scr[:, 1, :], op=OP.max
        )

    with tile.TileContext(nc) as tc:
        with (
            tc.tile_pool(name="const", bufs=1) as cp,
            tc.tile_pool(name="xp", bufs=xp_bufs) as xp,
            tc.tile_pool(name="x8p", bufs=2) as x8p,
            tc.tile_pool(name="scp", bufs=2) as scp,
            tc.tile_pool(name="xcp", bufs=4) as xcp,
            tc.tile_pool(name="smp", bufs=2) as smp,
            tc.tile_pool(name="atp", bufs=4) as atp,
            tc.tile_pool(name="spool", bufs=2, space="PSUM") as spool,
            tc.tile_pool(name="apsum", bufs=2, space="PSUM") as apsum,
        ):
            # weights go through the ACT-engine DGE queue so the first x-tile
            # load starts immediately on the SP queue
            id2 = cp.tile([128, 2, 128], f8)
            nc.scalar.dma_start(out=id2, in_=id2_d[:, :, :])
            w1sb = cp.tile([128, CT, TC], f16)
            nc.scalar.dma_start(
                out=w1sb, in_=w1_d.rearrange("(ct c) t -> c ct t", ct=CT)
            )
            b1sb = cp.tile([TC, 1], f32)
            nc.scalar.dma_start(out=b1sb, in_=b1_d[:, :])
            wsb = {}
            for nm, d in (("w2", w2_d), ("w3", w3_d), ("w4", w4_d)):
                t = cp.tile([TC, 2, C], f16, tag=f"wsb_{nm}")
                nc.scalar.dma_start(out=t, in_=d[:, :, :])
                wsb[nm] = t
            bsb = {}
            for nm, d in (("b2", b2_d), ("b3", b3_d), ("b4", b4_d)):
                t = cp.tile([128, CT, 1], f32, tag=f"bsb_{nm}")
                nc.scalar.dma_start(
                    out=t, in_=d.rearrange("(ct c) one -> c ct one", ct=CT)
                )
                bsb[nm] = t

            # PE observes id2 once so later matmuls carry one wait.
            warm = apsum.tile([128, 128], f32, tag="ap")
            nc.tensor.matmul(warm, id2[:, 0, :], id2[:, 0, :], start=True,
                             stop=True)

            # ---- loads: x16 row-halves first (trees gate on these); each
            # tile's fp8 copy (sums gate) is deferred behind the NEXT tile's
            # fp16 halves so DVE tree work is never starved early ----
            xt, x8t, xcs, psums = {}, {}, {}, {}
            for s in range(per):
                for ct in range(CT):
                    t = xp.tile([128, H, W], f16, tag="x")
                    for hh in range(2):
                        nc.sync.dma_start(
                            out=t[:, hh * HF : (hh + 1) * HF, :],
                            in_=x_d[
                                s, ct * 128 : (ct + 1) * 128,
                                hh * HF : (hh + 1) * HF, :,
                            ],
                        )
                    xt[(s, ct)] = t
                    t8 = x8p.tile([128, H, W], f8, tag="x8")
                    nc.sync.dma_start(
                        out=t8, in_=x8_d[s, ct * 128 : (ct + 1) * 128, :, :]
                    )
                    x8t[(s, ct)] = t8

            def pools(s, ct, emit_means):
                ch = xt[(s, ct)]
                x8 = x8t[(s, ct)]
                xcat = xcp.tile([128, 2, SS], f16, tag="xc")
                # max over w (DVE). Only the very first tile runs per
                # row-half (to start at half-transfer); later tiles' loads
                # are already ahead of DVE, so they use one full-height
                # tree with 11 fewer instructions.
                if s == 0 and ct == 0:
                    for hh in range(2):
                        scw = scp.tile([128, HF, W // G8], f16, tag="scw")
                        tree_w_rows(
                            ch, xcat[:, 1, hh * HF : (hh + 1) * HF], scw,
                            hh * HF, (hh + 1) * HF,
                        )
                else:
                    scwf = scp.tile([128, H, W // G8], f16, tag="scwf")
                    tree_w_rows(ch, xcat[:, 1, 0:H], scwf, 0, H)
                # max over h (DVE): first stage is row-half aligned
                sch = scp.tile([128, H // G8, W], f16, tag="sch")
                for hh in range(2):
                    tree_h_first(ch, sch, hh)
                tree_h_tail(xcat[:, 1, H : H + W], sch)
                # global max
                nc.vector.reduce_max(
                    out=xcat[:, 1, H + W : SS], in_=xcat[:, 1, 0:H], axis=AX.X
                )
                # sum over w first (its psum gates the means -> conv1 chain),
                # then sum over h (PE, DoubleRow fp8)
                psw = spool.tile([128, H], f32, tag="psw")
                wv = x8.rearrange("p h (g two) -> p two g h", two=2)
                gmax = W // 2
                for g in range(gmax):
                    nc.tensor.matmul(
                        psw, id2, wv[:, :, g, :],
                        start=(g == 0), stop=(g == gmax - 1), perf_mode=DR,
                    )
                psh = spool.tile([128, W], f32, tag="psh")
                hv = x8.rearrange("p (g two) w -> p two g w", two=2)
                gmax = H // 2
                for g in range(gmax):
                    nc.tensor.matmul(
                        psh, id2, hv[:, :, g, :],
                        start=(g == 0), stop=(g == gmax - 1), perf_mode=DR,
                    )
                xcs[(s, ct)] = xcat
                psums[(s, ct)] = (psh, psw)
                if emit_means:
                    means(s, ct)

            def means(s, ct):
                xcat = xcs[(s, ct)]
                psh, psw = psums[(s, ct)]
                acc = smp.tile([128, 1], f32, tag="acc")
                # mean-w and the global need only psw (an earlier PE block),
                # so just mean-h remains on the conv1 chain after the last
                # sum block lands
                nc.scalar.activation(
                    out=xcat[:, 0, 0:H], in_=psw, func=AF.Copy,
                    scale=1.0 / W, accum_out=acc,
                )
                nc.scalar.activation(
                    out=xcat[:, 0, H + W : SS], in_=acc, func=AF.Copy,
                    scale=1.0 / H,
                )
                nc.scalar.activation(
                    out=xcat[:, 0, H : H + W], in_=psh, func=AF.Copy,
                    scale=1.0 / H,
                )

            def conv1(s):
                """conv1 matmuls + bias (PE/ACT only, no DVE)."""
                xh = smp.tile([TC, 2, SS], f16, tag=f"xh{s}")
                for k in range(2):
                    yp = apsum.tile([TC, SS], f32, tag="y")
                    for ct in range(CT):
                        nc.tensor.matmul(
                            yp, w1sb[:, ct, :], xcs[(s, ct)][:, k, :],
                            start=(ct == 0), stop=(ct == CT - 1),
                        )
                    nc.scalar.add(out=xh[:, k, :], in_=yp, add=b1sb)
                return xh

            def hswish(xh):
                """the DVE part of h_swish, emitted late to avoid stalls."""
                for k in range(2):
                    xhk = xh[:, k, :]
                    u = smp.tile([TC, SS], f16, tag="u")
                    nc.vector.tensor_scalar(
                        out=u, in0=xhk,
                        scalar1=-3.0, scalar2=3.0, op0=OP.max, op1=OP.min,
                    )
                    nc.vector.scalar_tensor_tensor(
                        out=xhk, in0=u, scalar=3.0, in1=xhk,
                        op0=OP.add, op1=OP.mult,
                    )

            def attention(s, xh):
                """per-ct attention vectors: att (a_h|a_w), sf=a_h*a_c (f32),
                s2 = pair-duplicated sf (f16). Emission is interleaved across
                the channel tiles phase-by-phase so the 2-deep attention psum
                ring's reuse dependency is two sigmoids back, not one - the
                matmuls then dispatch in a single PE window."""
                att, ac = {}, {}
                for ct in range(CT):
                    att_t = atp.tile([128, SS], f16, tag="att")
                    att[ct] = att_t
                for wk, bk, lo, hi in (("w2", "b2", 0, H),
                                       ("w3", "b3", H, H + W),
                                       ("w4", "b4", H + W, SS)):
                    for ct in range(CT):
                        pp = apsum.tile([128, hi - lo], f32, tag="ap")
                        for k in range(2):
                            nc.tensor.matmul(
                                pp, wsb[wk][:, k, ct * 128 : (ct + 1) * 128],
                                xh[:, k, lo:hi], start=(k == 0), stop=(k == 1),
                            )
                        if wk == "w4":
                            ac_t = atp.tile([128, 1], f32, tag="ac")
                            ac[ct] = ac_t
                            dst = ac_t
                        else:
                            dst = att[ct][:, lo:hi]
                        nc.scalar.activation(
                            out=dst, in_=pp, func=AF.Sigmoid,
                            bias=bsb[bk][:, ct, :], scale=1.0,
                        )
                out = {}
                for ct in range(CT):
                    # s = a_h * a_c: f32 row for ACT row-scales + f16 pairs
                    # for the DVE TT (packed innermost keeps 2x mode)
                    sf = atp.tile([128, H], f32, tag="sf")
                    nc.scalar.activation(
                        out=sf, in_=att[ct][:, 0:H], func=AF.Copy, scale=ac[ct]
                    )
                    s2 = atp.tile([128, RA, 2], f16, tag="s2")
                    for half in range(2):
                        nc.scalar.activation(
                            out=s2[:, :, half], in_=att[ct][:, 0:RA],
                            func=AF.Copy, scale=ac[ct],
                        )
                    s2b = atp.tile([128, RA, 2], f16, tag="s2b")
                    for half in range(2):
                        nc.scalar.activation(
                            out=s2b[:, :, half], in_=att[ct][:, HF : HF + RA],
                            func=AF.Copy, scale=ac[ct],
                        )
                    out[ct] = (att[ct], sf, {0: s2, 1: s2b})
                return out

            def applies(s, avs):
                for ct in range(CT):
                    att, sf, s2h = avs[ct]
                    ch = xt[(s, ct)]
                    for hh in range(2):
                        r0 = hh * HF
                        chp = ch[:, r0 : r0 + HF, :]
                        awb = att[:, H : H + W].unsqueeze(1).to_broadcast(
                            [128, HF, W]
                        )
                        nc.vector.tensor_tensor(
                            out=chp, in0=chp, in1=awb, op=OP.mult
                        )
                        # s-multiply: first RA rows on DVE (pair TT), the
                        # rest on ACT (per-row per-partition scale)
                        chd = ch[:, r0 : r0 + RA, :]
                        xv = chd.rearrange("p h (w1 w0) -> p h w1 w0", w0=2)
                        s2v = s2h[hh].unsqueeze(2).to_broadcast(
                            [128, RA, W // 2, 2]
                        )
                        nc.vector.tensor_tensor(
                            out=xv, in0=xv, in1=s2v, op=OP.mult
                        )
                        for r in range(r0 + RA, r0 + HF):
                            row = ch[:, r, :]
                            nc.scalar.activation(
                                out=row, in_=row, func=AF.Copy,
                                scale=sf[:, r : r + 1],
                            )
                        nc.sync.dma_start(
                            out=o_d[
                                s, ct * 128 : (ct + 1) * 128, r0 : r0 + HF, :
                            ],
                            in_=chp,
                        )

            # ---- emission schedule (shapes the per-engine priority order) ----
            pools(0, 0, emit_means=True)
            pools(0, 1, emit_means=True)
            xh0 = conv1(0)
            pools(1, 0, emit_means=False)
            pools(1, 1, emit_means=False)
            hswish(xh0)
            av0 = attention(0, xh0)
            means(1, 0)
            applies(0, av0)
            means(1, 1)
            xh1 = conv1(1)
            hswish(xh1)
            av1 = attention(1, xh1)
            applies(1, av1)

    if split_waits:
        _split_excess_waits(nc)
    return nc


def prep_weights(w1, b1, bn_gamma, bn_beta, bn_mean, bn_var, w2, b2, w3, b3, w4, b4):
    import ml_dtypes

    inv = (bn_gamma / np.sqrt(bn_var + EPS)).astype(np.float32)
    w1f = (w1 * inv[:, None]).astype(np.float32)          # [TC, C]
    b1f = ((b1 - bn_mean) * inv + bn_beta).astype(np.float32)
    def pack(wk):  # [C, TC, 2] -> [TC, 2, C], with the h_swish /6 folded in
        return np.ascontiguousarray(wk.transpose(1, 2, 0) / 6.0).astype(np.float16)
    id2 = np.zeros((128, 2, 128), dtype=ml_dtypes.float8_e4m3)
    idx = np.arange(128)
    id2[idx, 0, idx] = 1.0
    id2[idx, 1, idx] = 1.0
    return dict(
        id2=id2,
        w1t=np.ascontiguousarray(w1f.T).astype(np.float16),   # [C, TC]
        b1f=b1f.reshape(TC, 1),
        w2t=pack(w2), w3t=pack(w3), w4t=pack(w4),
        b2r=b2.reshape(C, 1).astype(np.float32),
        b3r=b3.reshape(C, 1).astype(np.float32),
        b4r=b4.reshape(C, 1).astype(np.float32),
    )


_NC_CACHE = {}


def _get_nc():
    if "nc" not in _NC_CACHE:
        _NC_CACHE["nc"] = build_nc()
    return _NC_CACHE["nc"]


def make_in_maps(x, w1, b1, bn_gamma, bn_beta, bn_mean, bn_var,
                 w2, b2, w3, b3, w4, b4):
    import ml_dtypes

    x16 = np.asarray(x).astype(np.float16)
    x8 = x16.astype(ml_dtypes.float8_e4m3)
    wmap = prep_weights(
        np.asarray(w1, np.float32), np.asarray(b1, np.float32),
        np.asarray(bn_gamma, np.float32), np.asarray(bn_beta, np.float32),
        np.asarray(bn_mean, np.float32), np.asarray(bn_var, np.float32),
        np.asarray(w2, np.float32), np.asarray(b2, np.float32),
        np.asarray(w3, np.float32), np.asarray(b3, np.float32),
        np.asarray(w4, np.float32), np.asarray(b4, np.float32),
    )
    return [
        {
            "x16": np.ascontiguousarray(x16[i * PER : (i + 1) * PER]),
            "x8": np.ascontiguousarray(x8[i * PER : (i + 1) * PER]),
            **wmap,
        }
        for i in range(NCORES)
    ]


def gather_out(results):
    return np.concatenate(
        [results[i]["out"] for i in range(NCORES)], axis=0
    ).astype(np.float32)


def kernel(x, w1, b1, bn_gamma, bn_beta, bn_mean, bn_var, w2, b2, w3, b3, w4, b4):
    from concourse.bass_utils import run_bass_kernel_spmd

    nc = _get_nc()
    in_maps = make_in_maps(x, w1, b1, bn_gamma, bn_beta, bn_mean, bn_var,
                           w2, b2, w3, b3, w4, b4)
    res = run_bass_kernel_spmd(nc, in_maps, core_ids=list(range(NCORES)))
    return gather_out(res.results)
